# revision 1
# baseline (speedup 1.0000x reference)
"""Trainium2 Bass kernel for the nn_Block dense-transformer variant.

Contract: kernel(**inputs) takes FULL unsharded inputs (as in reference.setup_inputs)
and returns the FULL (N, S, E) float32 output.

Math (faithful to the reference's unusual einsums):
  q/k/v projections -> energy[n,qh,kh,s] = sum_d q[n,qh,s,d]*k[n,kh,s,d]  (per-position
  head-Gram matrices), softmax over s (the sequence!), and the output einsum factorizes:
     out[n,qh,s,d] = A[n,qh,s] * Vs[n,s,d]
  with A = sum_kh softmax_s(energy/8) and Vs = sum_vh v.  The (n,qh,s,d) tensor is then
  C-order reshaped to (n, s', e') with s' = qh*128 + s//16, e' = (s%16)*64 + (d) and fed
  through Wo + residual + LN1 + FFN + LN2.

Sharding: 8 cores, core c handles batch n=c//2 and head-half hs=c%2 (qh in [8hs, 8hs+8)),
which makes output rows t' in [1024hs, 1024hs+1024) — fully local, zero collectives.
"""

import sys

sys.path.insert(0, "/opt/trn_rl_repo")

from contextlib import ExitStack

import numpy as np
import ml_dtypes

import concourse.bass as bass
from concourse import bacc
import concourse.mybir as mybir
import concourse.tile as tile
from concourse.bass import ts
from concourse.bass_utils import run_bass_kernel_spmd

F32 = mybir.dt.float32
F32R = mybir.dt.float32r
BF16 = mybir.dt.bfloat16
AF = mybir.ActivationFunctionType
ALU = mybir.AluOpType

E = 1024
H = 16
D = 64
F = 4096
NB = 4
S = 2048
TLOC = 1024  # output rows per core
EPS = 1e-5
NCORES = 8

_BF = ml_dtypes.bfloat16

_nc_cache = {}


# ---------------------------------------------------------------------------
# device program
# ---------------------------------------------------------------------------

def _declare(nc):
    dp = nc.declare_dram_parameter
    t = {}
    t["xth"] = dp("xth", [128, 8, S], BF16, isOutput=False)        # x[n].T chunked, bf16
    t["xtr"] = dp("xtr", [128, 8, TLOC], F32, isOutput=False)      # residual slice of x
    t["wqr"] = dp("wqr", [8, 128, 8, 128], BF16, isOutput=False)  # [qh][p, ek, i] doubled head block
    t["wk"] = dp("wk", [128, 8, E], BF16, isOutput=False)
    t["wvsl"] = dp("wvsl", [128, 8, 128], BF16, isOutput=False)
    t["wvsr"] = dp("wvsr", [128, 8, 128], BF16, isOutput=False)
    t["wo"] = dp("wo", [128, 8, E], BF16, isOutput=False)
    t["w1"] = dp("w1", [32, 128, 8, 128], F32R, isOutput=False)    # [fc][p, ek, c]
    t["w2"] = dp("w2", [8, 2, 128, 16, 128], F32R, isOutput=False)  # [mc][fch][p, fc, c]
    t["maske"] = dp("maske", [128, 8, 32], BF16, isOutput=False)
    t["maskq"] = dp("maskq", [128, 2, 8], F32, isOutput=False)
    t["bc2"] = dp("bc2", [2, 128], BF16, isOutput=False)
    t["onesk"] = dp("onesk", [128, 1], F32R, isOutput=False)
    t["onesm"] = dp("onesm", [1, 128], F32R, isOutput=False)
    t["bo"] = dp("bo", [128, 8], F32, isOutput=False)
    t["bf1"] = dp("bf1", [128, 32], F32, isOutput=False)
    t["bf2"] = dp("bf2", [128, 8], F32, isOutput=False)
    t["ln1w"] = dp("ln1w", [128, 8], F32, isOutput=False)
    t["ln1b"] = dp("ln1b", [128, 8], F32, isOutput=False)
    t["ln2w"] = dp("ln2w", [128, 8], F32, isOutput=False)
    t["ln2b"] = dp("ln2b", [128, 8], F32, isOutput=False)
    t["out"] = dp("out", [E, TLOC], F32, isOutput=True)            # y2 transposed [e, t']
    return t


def _emit_layernorm(nc, ctx, tc, pools, src, dst_write, wcol, bcol, eps_sb, tag):
    """LayerNorm over the feature (partition-chunk) axis.

    src: SBUF tile [128, 8, 1024] F32R (feature-partition layout, 8 E-chunks).
    dst_write(mc, w2) -> AP [128, 512] to write the normalized output into.
    wcol/bcol: [128, 8] F32 per-feature affine params.
    """
    spool = pools["ln_sbuf"]
    rpool = pools["ln_row"]
    pps = pools["ln_psum"]
    for w2 in range(2):
        s_ps = pps.tile([1, 512], F32, tag=f"{tag}_s")
        q_ps = pps.tile([1, 512], F32, tag=f"{tag}_q")
        for mc in range(8):
            sl = src[:, mc, ts(w2, 512)]
            nc.tensor.matmul(s_ps[:], pools["onesk"][:], sl, start=(mc == 0), stop=(mc == 7))
        for mc in range(8):
            sq = spool.tile([128, 512], F32R, tag=f"{tag}_sq")
            nc.scalar.activation(out=sq[:], in_=src[:, mc, ts(w2, 512)].bitcast(F32), func=AF.Square)
            nc.tensor.matmul(q_ps[:], pools["onesk"][:], sq[:], start=(mc == 0), stop=(mc == 7))
        mean = rpool.tile([1, 512], F32R, tag=f"{tag}_mean")
        msq = rpool.tile([1, 512], F32, tag=f"{tag}_msq")
        nc.scalar.activation(out=mean[:], in_=s_ps[:], func=AF.Copy, scale=1.0 / E)
        nc.scalar.activation(out=msq[:], in_=q_ps[:], func=AF.Copy, scale=1.0 / E)
        m2 = rpool.tile([1, 512], F32, tag=f"{tag}_m2")
        nc.vector.tensor_mul(m2[:], mean[:].bitcast(F32), mean[:].bitcast(F32))
        var = rpool.tile([1, 512], F32, tag=f"{tag}_var")
        nc.vector.tensor_tensor(var[:], msq[:], m2[:], ALU.subtract)
        sd = rpool.tile([1, 512], F32, tag=f"{tag}_sd")
        nc.scalar.activation(out=sd[:], in_=var[:], func=AF.Sqrt, bias=eps_sb[:])
        rs = rpool.tile([1, 512], F32R, tag=f"{tag}_rs")
        with nc.allow_low_precision(reason="f32r is ample for the LN rstd row"):
            nc.vector.reciprocal(rs[:], sd[:])
        # broadcast mean and rs across 128 partitions via K=1 matmuls
        mb_ps = pps.tile([128, 512], F32, tag=f"{tag}_mb")
        rb_ps = pps.tile([128, 512], F32, tag=f"{tag}_rb")
        nc.tensor.matmul(mb_ps[:], pools["onesm"][:], mean[:], start=True, stop=True)
        nc.tensor.matmul(rb_ps[:], pools["onesm"][:], rs[:], start=True, stop=True)
        for mc in range(8):
            t1 = spool.tile([128, 512], F32, tag=f"{tag}_t1")
            nc.vector.tensor_tensor(t1[:], src[:, mc, ts(w2, 512)].bitcast(F32), mb_ps[:], ALU.subtract)
            t2 = spool.tile([128, 512], F32, tag=f"{tag}_t2")
            nc.vector.tensor_mul(t2[:], t1[:], rb_ps[:])
            nc.vector.tensor_scalar(
                out=dst_write(mc, w2),
                in0=t2[:],
                scalar1=wcol[:, mc : mc + 1],
                scalar2=bcol[:, mc : mc + 1],
                op0=ALU.mult,
                op1=ALU.add,
            )


def _emit(nc, t, reps=1):
    with tile.TileContext(nc) as tc:
      for _rep in range(reps):
        with ExitStack() as octx:
          pool_const = octx.enter_context(tc.tile_pool(name="const", bufs=1))

          # ---- constants ----
          maske_sb = pool_const.tile([128, 8, 32], BF16, tag="maske")
          nc.sync.dma_start(maske_sb[:], t["maske"].ap()[:, :, :])
          maskq_sb = pool_const.tile([128, 2, 8], F32, tag="maskq")
          nc.sync.dma_start(maskq_sb[:], t["maskq"].ap()[:, :, :])
          bc2_sb = pool_const.tile([2, 128], BF16, tag="bc2")
          nc.sync.dma_start(bc2_sb[:], t["bc2"].ap()[:, :])
          onesk_sb = pool_const.tile([128, 1], F32R, tag="onesk")
          nc.sync.dma_start(onesk_sb[:], t["onesk"].ap()[:, :])
          onesm_sb = pool_const.tile([1, 128], F32R, tag="onesm")
          nc.sync.dma_start(onesm_sb[:], t["onesm"].ap()[:, :])
          cols = {}
          for nm in ("bo", "bf1", "bf2", "ln1w", "ln1b", "ln2w", "ln2b"):
              sh = [128, 32] if nm == "bf1" else [128, 8]
              cols[nm] = pool_const.tile(sh, F32, tag=nm, name=nm)
              nc.sync.dma_start(cols[nm][:], t[nm].ap()[:, :])
          eps_sb = pool_const.tile([1, 1], F32, tag="eps")
          nc.vector.memset(eps_sb[:], EPS)

          pool_y1 = octx.enter_context(tc.tile_pool(name="y1p", bufs=1))
          y1_sb = pool_y1.tile([128, 8, TLOC], F32R, tag="y1")

          with ExitStack() as actx:
              pool_att = actx.enter_context(tc.tile_pool(name="att", bufs=1))
              u_sb = pool_att.tile([128, 2, S], F32R, tag="u")
              vsh_sb = pool_att.tile([128, S], F32, tag="vsh")
              dsum_sb = pool_att.tile([128, 8], F32, tag="dsum")
              dden_sb = pool_att.tile([128, 2], F32, tag="dden")
              rec_sb = pool_att.tile([128, 2], F32, tag="rec")
              v_sb = pool_att.tile([128, 2, 8], F32R, tag="vv")
              a_sb = pool_att.tile([8, 16, 128], BF16, tag="aa")  # [qh, r, sb]
              arow_sb = pool_att.tile([2, 8, 8, 128], BF16, tag="arow")  # [j, qh, pc, sb]

              # ================= phases B/C/D: projections + energy =================
              with ExitStack() as pctx:
                  pool_x = pctx.enter_context(tc.tile_pool(name="px", bufs=1))
                  pool_wq = pctx.enter_context(tc.tile_pool(name="pwq", bufs=2))
                  pool_kt = pctx.enter_context(tc.tile_pool(name="pkt", bufs=1))
                  pool_str = pctx.enter_context(tc.tile_pool(name="pstr", bufs=3))
                  ps_pp = pctx.enter_context(tc.tile_pool(name="pspp", bufs=4, space="PSUM"))
                  ps_e = pctx.enter_context(tc.tile_pool(name="pse", bufs=4, space="PSUM"))

                  xth_sb = pool_x.tile([128, 8, S + 1], BF16, tag="xth")
                  for w in range(4):
                      nc.sync.dma_start(
                          xth_sb[:, :, ts(w, 512)], t["xth"].ap()[:, :, ts(w, 512)]
                      )
                  nc.vector.memset(xth_sb[:, :, S : S + 1], 0.0)
                  wk_sb = pool_x.tile([128, 8, E], BF16, tag="wk")
                  nc.sync.dma_start(wk_sb[:], t["wk"].ap()[:, :, :])
                  wvsl_sb = pool_x.tile([128, 8, 128], BF16, tag="wvsl")
                  nc.sync.dma_start(wvsl_sb[:], t["wvsl"].ap()[:, :, :])
                  wvsr_sb = pool_x.tile([128, 8, 128], BF16, tag="wvsr")
                  nc.sync.dma_start(wvsr_sb[:], t["wvsr"].ap()[:, :, :])

                  kt_sb = pool_kt.tile([128, 8, S], BF16, tag="kt")

                  # ---- B: k projection (all 16 heads); ki outer so lhsT is reused ----
                  for ek in range(8):
                      kps = [ps_pp.tile([128, 512], F32, tag="pp", name=f"kp{ek}_{i}") for i in range(4)]
                      for ki in range(8):
                          for w in range(4):
                              nc.tensor.matmul(
                                  kps[w][:],
                                  wk_sb[:, ki, ts(ek, 128)],
                                  xth_sb[:, ki, ts(w, 512)],
                                  start=(ki == 0),
                                  stop=(ki == 7),
                              )
                      for w in range(4):
                          nc.scalar.copy(kt_sb[:, ek, ts(w, 512)], kps[w][:])

                  # ---- D: shifted Vs projection (vsh[i, s] = Vs[s + i//64, i%64]) ----
                  vps = [ps_pp.tile([128, 512], F32, tag="pp", name=f"vp{i}") for i in range(4)]
                  for ki in range(8):
                      for w in range(4):
                          nc.tensor.matmul(
                              vps[w][:],
                              wvsl_sb[:, ki, :],
                              xth_sb[:, ki, ts(w, 512)],
                              start=(ki == 0),
                              stop=False,
                          )
                  for ki in range(8):
                      for w in range(4):
                          nc.tensor.matmul(
                              vps[w][:],
                              wvsr_sb[:, ki, :],
                              xth_sb[:, ki, w * 512 + 1 : w * 512 + 513],
                              start=False,
                              stop=(ki == 7),
                          )
                  for w in range(4):
                      nc.scalar.copy(vsh_sb[:, ts(w, 512)], vps[w][:])

                  # ---- C: qrep2 + products + energy block-reduce + exp ----
                  for h in range(2):
                      e_tiles = [ps_e.tile([128, 512], F32, tag="e", name=f"e{h}_{i}") for i in range(4)]
                      for jq in range(4):
                          qh = 4 * h + jq
                          wqr_t = pool_wq.tile([128, 8, 128], BF16, tag="wqrt")
                          nc.sync.dma_start(wqr_t[:], t["wqr"].ap()[qh, :, :, :])
                          qr_sb = pool_str.tile([128, S], BF16, tag="qr")
                          qps = [ps_pp.tile([128, 512], F32, tag="pp", name=f"qp{qh}_{i}") for i in range(4)]
                          for ki in range(8):
                              for w in range(4):
                                  nc.tensor.matmul(
                                      qps[w][:],
                                      wqr_t[:, ki, :],
                                      xth_sb[:, ki, ts(w, 512)],
                                      start=(ki == 0),
                                      stop=(ki == 7),
                                  )
                          for w in range(4):
                              nc.scalar.copy(qr_sb[:, ts(w, 512)], qps[w][:])
                          for ek in range(8):
                              z = pool_str.tile([128, S], BF16, tag="z")
                              nc.vector.tensor_mul(z[:], kt_sb[:, ek, :], qr_sb[:])
                              for w in range(4):
                                  nc.tensor.matmul(
                                      e_tiles[w][32 * jq : 32 * jq + 32, :],
                                      maske_sb[:, ek, :],
                                      z[:, ts(w, 512)],
                                      start=(ek == 0),
                                      stop=(ek == 7),
                                      tile_position=(0, 32 * jq),
                                      skip_group_check=True,
                                  )
                      for w in range(4):
                          nc.scalar.activation(
                              out=u_sb[:, h, ts(w, 512)],
                              in_=e_tiles[w][:],
                              func=AF.Exp,
                              scale=0.125,
                              accum_out=dsum_sb[:, 4 * h + w : 4 * h + w + 1],
                          )

              # ================= phase E: denominators + A =================
              with ExitStack() as ectx:
                  ps_a = ectx.enter_context(tc.tile_pool(name="psa", bufs=2, space="PSUM"))
                  for h in range(2):
                      nc.vector.reduce_sum(
                          out=dden_sb[:, h : h + 1],
                          in_=dsum_sb[:, ts(h, 4)],
                          axis=mybir.AxisListType.X,
                      )
                      nc.vector.reciprocal(rec_sb[:, h : h + 1], dden_sb[:, h : h + 1])
                      nc.vector.tensor_scalar_mul(
                          out=v_sb[:, h, :], in0=maskq_sb[:, h, :], scalar1=rec_sb[:, h : h + 1]
                      )
                  for w in range(4):
                      a_ps = ps_a.tile([8, 512], F32, tag="a")
                      nc.tensor.matmul(a_ps[:], v_sb[:, 0, :], u_sb[:, 0, ts(w, 512)], start=True, stop=False)
                      nc.tensor.matmul(a_ps[:], v_sb[:, 1, :], u_sb[:, 1, ts(w, 512)], start=False, stop=True)
                      # scatter-write A window into [qh, r, sb] layout:
                      # in t-order (sb outer, r inner) -> out dims (sb, r)
                      nc.scalar.copy(
                          a_sb[:, :, 32 * w : 32 * w + 32].transpose([0, 2, 1]), a_ps[:]
                      )

                  # A gather: arow[j, qh, pc, sb] = a_sb[qh, 2*pc+j, sb]
                  with nc.allow_non_contiguous_dma(reason="tiny A gather"):
                      for j in range(2):
                          nc.sync.dma_start(
                              arow_sb[j : j + 1, :, :, :],
                              a_sb[:, j::2, :],
                          )

              # ================= phase F: OFT build =================
              with ExitStack() as fctx:
                  pool_oft = fctx.enter_context(tc.tile_pool(name="poft", bufs=1))
                  oft_sb = pool_oft.tile([128, 8, TLOC], BF16, tag="oft")
                  with ExitStack() as ffctx:
                      ps_fa = ffctx.enter_context(tc.tile_pool(name="psfa", bufs=2, space="PSUM"))
                      for pc in range(8):
                          for w2 in range(2):
                              fa_ps = ps_fa.tile([128, 512], F32, tag="fa")
                              nc.tensor.matmul(
                                  fa_ps[:],
                                  bc2_sb[:],
                                  arow_sb[:, ts(w2, 4), pc, :],
                                  start=True,
                                  stop=True,
                              )
                              nc.vector.tensor_mul(
                                  oft_sb[:, pc, ts(w2, 512)].rearrange("p (a b) -> p a b", a=4),
                                  fa_ps[:].rearrange("p (a b) -> p a b", a=4),
                                  vsh_sb[:, 2 * pc :: 16].unsqueeze(1).broadcast_to([128, 4, 128]),
                              )

                  # ================= phase G: Wo + residual + LN1 =================
                  with ExitStack() as gctx:
                          pool_g = gctx.enter_context(tc.tile_pool(name="pg", bufs=1))
                          pool_gx = gctx.enter_context(tc.tile_pool(name="pgx", bufs=2))
                          pool_gt = gctx.enter_context(tc.tile_pool(name="pgt", bufs=2))
                          pool_grow = gctx.enter_context(tc.tile_pool(name="pgrow", bufs=1))
                          ps_g = gctx.enter_context(tc.tile_pool(name="psg", bufs=4, space="PSUM"))
                          ps_ln1 = gctx.enter_context(tc.tile_pool(name="psln1", bufs=1, space="PSUM"))

                          wo_sb = pool_g.tile([128, 8, E], BF16, tag="wo")
                          nc.sync.dma_start(wo_sb[:], t["wo"].ap()[:, :, :])
                          y1pre_sb = pool_g.tile([128, 8, TLOC], F32R, tag="y1pre")

                          for mc in range(8):
                              xtr_t = pool_gx.tile([128, TLOC], F32, tag="xtrt", name=f"xtr{mc}")
                              nc.sync.dma_start(xtr_t[:], t["xtr"].ap()[:, mc, :])
                              at_tiles = [
                                  ps_g.tile([128, 512], F32, tag="at", name=f"at{mc}_{i}")
                                  for i in range(2)
                              ]
                              for pc in range(8):
                                  for w2 in range(2):
                                      nc.tensor.matmul(
                                          at_tiles[w2][:],
                                          wo_sb[:, pc, ts(mc, 128)],
                                          oft_sb[:, pc, ts(w2, 512)],
                                          start=(pc == 0),
                                          stop=(pc == 7),
                                      )
                              for w2 in range(2):
                                  attn = pool_gt.tile([128, 512], F32, tag="attn", name=f"attn{mc}_{w2}")
                                  nc.scalar.activation(
                                      out=attn[:], in_=at_tiles[w2][:], func=AF.Identity, bias=cols["bo"][:, mc : mc + 1]
                                  )
                                  nc.vector.tensor_add(
                                      y1pre_sb[:, mc, ts(w2, 512)], attn[:], xtr_t[:, ts(w2, 512)]
                                  )

                          ln_pools = {
                              "ln_sbuf": pool_gt,
                              "ln_row": pool_grow,
                              "ln_psum": ps_ln1,
                              "onesk": onesk_sb,
                              "onesm": onesm_sb,
                          }
                          _emit_layernorm(
                              nc, gctx, tc, ln_pools,
                              src=y1pre_sb,
                              dst_write=lambda mc, w2: y1_sb[:, mc, ts(w2, 512)],
                              wcol=cols["ln1w"], bcol=cols["ln1b"], eps_sb=eps_sb, tag="ln1",
                          )
          # ================= phase H: FFN + LN2 + out =================
          with ExitStack() as hctx:
              pool_h = hctx.enter_context(tc.tile_pool(name="ph", bufs=1))
              pool_w1 = hctx.enter_context(tc.tile_pool(name="pw1", bufs=2))
              pool_w2 = hctx.enter_context(tc.tile_pool(name="pw2", bufs=2))
              pool_ht = hctx.enter_context(tc.tile_pool(name="pht", bufs=2))
              pool_hrow = hctx.enter_context(tc.tile_pool(name="phrow", bufs=1))
              pool_y2 = hctx.enter_context(tc.tile_pool(name="py2", bufs=2))
              ps_h = hctx.enter_context(tc.tile_pool(name="psh", bufs=2, space="PSUM"))
              ps_ln2 = hctx.enter_context(tc.tile_pool(name="psln2", bufs=1, space="PSUM"))

              h_sb = pool_h.tile([128, 32, 512], F32R, tag="hh")
              ffpre_sb = pool_h.tile([128, 8, TLOC], F32R, tag="ffpre")

              for w2 in range(2):
                  for fc in range(32):
                      w1t = pool_w1.tile([128, 8, 128], F32R, tag="w1t")
                      nc.sync.dma_start(w1t[:], t["w1"].ap()[fc, :, :, :])
                      h_ps = ps_h.tile([128, 512], F32, tag="h")
                      for ki in range(8):
                          nc.tensor.matmul(
                              h_ps[:],
                              w1t[:, ki, :],
                              y1_sb[:, ki, ts(w2, 512)],
                              start=(ki == 0),
                              stop=(ki == 7),
                          )
                      nc.scalar.activation(
                          out=h_sb[:, fc, :], in_=h_ps[:], func=AF.Relu, bias=cols["bf1"][:, fc : fc + 1]
                      )
                  for mc in range(8):
                      ff_ps = ps_h.tile([128, 512], F32, tag="ff")
                      for fch in range(2):
                          w2t = pool_w2.tile([128, 16, 128], F32R, tag="w2t")
                          nc.sync.dma_start(w2t[:], t["w2"].ap()[mc, fch, :, :, :])
                          for fi in range(16):
                              fc = fch * 16 + fi
                              nc.tensor.matmul(
                                  ff_ps[:],
                                  w2t[:, fi, :],
                                  h_sb[:, fc, :],
                                  start=(fc == 0),
                                  stop=(fc == 31),
                              )
                      fft = pool_ht.tile([128, 512], F32, tag="fft")
                      nc.scalar.activation(
                          out=fft[:], in_=ff_ps[:], func=AF.Identity, bias=cols["bf2"][:, mc : mc + 1]
                      )
                      nc.vector.tensor_add(
                          ffpre_sb[:, mc, ts(w2, 512)], fft[:], y1_sb[:, mc, ts(w2, 512)].bitcast(F32)
                      )

              ln_pools = {
                  "ln_sbuf": pool_ht,
                  "ln_row": pool_hrow,
                  "ln_psum": ps_ln2,
                  "onesk": onesk_sb,
                  "onesm": onesm_sb,
              }
              out_ap = t["out"].ap()

              written = []

              def dst2(mc, w2):
                  wtile = pool_y2.tile([128, 512], F32, tag="y2out", name=f"y2_{mc}_{w2}")
                  written.append((mc, w2, wtile))
                  return wtile[:]

              _emit_layernorm(
                  nc, hctx, tc, ln_pools,
                  src=ffpre_sb,
                  dst_write=dst2,
                  wcol=cols["ln2w"], bcol=cols["ln2b"], eps_sb=eps_sb, tag="ln2",
              )
              for mc, w2, wtile in written:
                  nc.sync.dma_start(out_ap[ts(mc, 128), ts(w2, 512)], wtile[:])


def build_nc(reps=1):
    key = f"nc{reps}"
    if key in _nc_cache:
        return _nc_cache[key]
    nc = bacc.Bacc("TRN2", target_bir_lowering=False, debug=False, num_devices=NCORES)
    t = _declare(nc)
    _emit(nc, t, reps=reps)
    nc.compile()
    _nc_cache[key] = nc
    return nc


# ---------------------------------------------------------------------------
# host side
# ---------------------------------------------------------------------------

def _chunk_pT(a):
    """[R, E] -> [128, E//128, R]: out[p, ek, r] = a[r, ek*128+p]."""
    r, e = a.shape
    return np.ascontiguousarray(a.reshape(r, e // 128, 128).transpose(2, 1, 0))


def _cols(b, nchunk):
    """[C] -> [128, nchunk]: out[p, c] = b[c*128+p]."""
    return np.ascontiguousarray(b.reshape(nchunk, 128).T)


def prepare_inputs(x, Wq, Wk, Wv, Wo, bo, ln1_w, ln1_b, ln2_w, ln2_b, W1, bf1, W2, bf2):
    """Build the 8 per-core input maps."""
    x = np.asarray(x, dtype=np.float32)
    Wq = np.asarray(Wq, dtype=np.float32)
    Wk = np.asarray(Wk, dtype=np.float32)
    Wv = np.asarray(Wv, dtype=np.float32)
    Wo = np.asarray(Wo, dtype=np.float32)
    W1 = np.asarray(W1, dtype=np.float32)
    W2 = np.asarray(W2, dtype=np.float32)
    bo = np.asarray(bo, dtype=np.float32)
    bf1 = np.asarray(bf1, dtype=np.float32)
    bf2 = np.asarray(bf2, dtype=np.float32)
    ln1_w = np.asarray(ln1_w, dtype=np.float32)
    ln1_b = np.asarray(ln1_b, dtype=np.float32)
    ln2_w = np.asarray(ln2_w, dtype=np.float32)
    ln2_b = np.asarray(ln2_b, dtype=np.float32)

    # shared (core-independent) tensors
    wk_h = _chunk_pT(Wk).astype(_BF)                      # [128, 8, 1024]
    Wvs = Wv.reshape(H, D, E).sum(axis=0)                 # [64, 1024]
    blk = _chunk_pT(Wvs)                                  # [128, 8, 64]
    z64 = np.zeros_like(blk)
    wvsl_h = np.concatenate([blk, z64], axis=2).astype(_BF)   # [128, 8, 128]
    wvsr_h = np.concatenate([z64, blk], axis=2).astype(_BF)
    wo_h = _chunk_pT(Wo).astype(_BF)
    w1_h = np.ascontiguousarray(
        W1.reshape(32, 128, 8, 128).transpose(0, 3, 2, 1)
    ).astype(np.float32)                                  # [32, 128, 8, 128]
    w2_h = np.ascontiguousarray(
        W2.reshape(8, 128, 2, 16, 128).transpose(0, 2, 4, 3, 1)
    ).astype(np.float32)                                  # [8, 2, 128, 16, 128]

    i128 = np.arange(128)
    maske = np.zeros((128, 8, 32), dtype=np.float32)
    for c in range(8):
        maske[i128, c, (2 * c + i128 // 64)] = 1.0
    maske = maske.astype(_BF)
    # half hh covers local qh' in [4*hh, 4*hh+4); pair (qh', kh) sits at
    # partition p = 32*(qh' % 4) + kh (kh < 16).  A-matmul column = qh'.
    maskq = np.zeros((128, 2, 8), dtype=np.float32)
    for hh in range(2):
        for p in range(128):
            if p % 32 < 16:
                maskq[p, hh, 4 * hh + p // 32] = 1.0
    bc2 = np.zeros((2, 128), dtype=np.float32)
    bc2[0, :64] = 1.0
    bc2[1, 64:] = 1.0
    bc2 = bc2.astype(_BF)
    onesk = np.ones((128, 1), dtype=np.float32)
    onesm = np.ones((1, 128), dtype=np.float32)

    shared = {
        "wk": wk_h, "wvsl": wvsl_h, "wvsr": wvsr_h, "wo": wo_h,
        "w1": w1_h, "w2": w2_h,
        "maske": maske, "maskq": maskq, "bc2": bc2,
        "onesk": onesk, "onesm": onesm,
        "bo": _cols(bo, 8), "bf1": _cols(bf1, 32), "bf2": _cols(bf2, 8),
        "ln1w": _cols(ln1_w, 8), "ln1b": _cols(ln1_b, 8),
        "ln2w": _cols(ln2_w, 8), "ln2b": _cols(ln2_b, 8),
    }

    in_maps = []
    for c in range(NCORES):
        n, hs = c // 2, c % 2
        xn = x[n]                                          # [2048, 1024]
        xth = _chunk_pT(xn).astype(_BF)                    # [128, 8, 2048]
        xtr = _chunk_pT(xn[hs * TLOC : (hs + 1) * TLOC])   # [128, 8, 1024] f32
        Wqh = Wq[hs * 512 : (hs + 1) * 512]                # [512, 1024]
        tmp = Wqh.reshape(8, D, 8, 128)                    # [qh, d, ek, p]
        dup = np.concatenate([tmp, tmp], axis=1)           # [qh, 128, ek, p]
        wqr = np.ascontiguousarray(dup.transpose(0, 3, 2, 1)).astype(_BF)  # [qh, 128, ek, 128]
        m = dict(shared)
        m["xth"] = xth
        m["xtr"] = xtr
        m["wqr"] = wqr
        in_maps.append(m)
    return in_maps


def assemble_output(results):
    out = np.empty((NB, S, E), dtype=np.float32)
    for c in range(NCORES):
        n, hs = c // 2, c % 2
        out[n, hs * TLOC : (hs + 1) * TLOC, :] = results[c]["out"].T
    return out


# ---------------------------------------------------------------------------
# cached PJRT runner (jit built once; shared weights replicated, not concatenated)
# ---------------------------------------------------------------------------

_PER_CORE = ("xth", "xtr", "wqr")  # inputs that differ per core


def _get_runner():
    if "runner" in _nc_cache:
        return _nc_cache["runner"]
    import jax
    from jax.sharding import Mesh, PartitionSpec
    from jax.experimental.shard_map import shard_map
    from concourse import bass2jax, mybir as _mb

    nc = build_nc()
    bass2jax.install_neuronx_cc_hook()

    partition_name = nc.partition_id_tensor.name if nc.partition_id_tensor else None
    in_names, out_names, out_avals, zero_shapes = [], [], [], []
    for alloc in nc.m.functions[0].allocations:
        if not isinstance(alloc, _mb.MemoryLocationSet):
            continue
        name = alloc.memorylocations[0].name
        if alloc.kind == "ExternalInput":
            if name != partition_name:
                in_names.append(name)
        elif alloc.kind == "ExternalOutput":
            out_names.append(name)
            shape = tuple(alloc.tensor_shape)
            dtype = _mb.dt.np(alloc.dtype)
            out_avals.append(jax.core.ShapedArray(shape, dtype))
            zero_shapes.append((shape, dtype))
    n_params = len(in_names)
    all_names = list(in_names) + out_names
    if partition_name is not None:
        all_names.append(partition_name)

    def _body(*args):
        operands = list(args)
        if partition_name is not None:
            operands.append(bass2jax.partition_id_tensor())
        outs = bass2jax._bass_exec_p.bind(
            *operands,
            out_avals=tuple(out_avals),
            in_names=tuple(all_names),
            out_names=tuple(out_names),
            lowering_input_output_aliases=(),
            sim_require_finite=True,
            sim_require_nnan=True,
            nc=nc,
        )
        return tuple(outs)

    devices = jax.devices()[:NCORES]
    mesh = Mesh(np.asarray(devices), ("core",))
    in_specs = tuple(
        PartitionSpec("core") if nm in _PER_CORE else PartitionSpec()
        for nm in in_names
    ) + (PartitionSpec("core"),) * len(out_names)
    out_specs = (PartitionSpec("core"),) * len(out_names)
    donate = tuple(range(n_params, n_params + len(out_names)))
    sharded = jax.jit(
        shard_map(_body, mesh=mesh, in_specs=in_specs, out_specs=out_specs, check_rep=False),
        donate_argnums=donate,
        keep_unused=True,
    )
    runner = {
        "jit": sharded,
        "in_names": in_names,
        "out_names": out_names,
        "zero_shapes": zero_shapes,
        "out_avals": out_avals,
        "mesh": mesh,
        "in_specs": in_specs,
    }
    _nc_cache["runner"] = runner
    return runner


def _pack_args(runner, in_maps):
    """Build the positional arg list for the jitted runner from per-core maps."""
    args = []
    for nm in runner["in_names"]:
        if nm in _PER_CORE:
            args.append(np.concatenate([np.asarray(m[nm]) for m in in_maps], axis=0))
        else:
            args.append(np.asarray(in_maps[0][nm]))
    return args


def _zero_outs(runner):
    return [
        np.zeros((NCORES * sh[0], *sh[1:]), dt) for sh, dt in runner["zero_shapes"]
    ]


def run_fast(in_maps):
    runner = _get_runner()
    args = _pack_args(runner, in_maps)
    outs = runner["jit"](*args, *_zero_outs(runner))
    res = []
    for c in range(NCORES):
        m = {}
        for i, nm in enumerate(runner["out_names"]):
            sh = runner["out_avals"][i].shape
            m[nm] = np.asarray(outs[i]).reshape(NCORES, *sh)[c]
        res.append(m)
    return res


def kernel(**inputs):
    in_maps = prepare_inputs(**inputs)
    return assemble_output(run_fast(in_maps))


if __name__ == "__main__":
    # quick self-build check
    nc = build_nc()
    print("built ok")



# revision 28
# speedup vs baseline: 1.3837x; 1.3837x over previous
"""Trainium2 Bass kernel for the nn_Block dense-transformer variant.

Contract: kernel(**inputs) takes FULL unsharded inputs (as in reference.setup_inputs)
and returns the FULL (N, S, E) float32 output.

Math (faithful to the reference's unusual einsums):
  q/k/v projections -> energy[n,qh,kh,s] = sum_d q[n,qh,s,d]*k[n,kh,s,d]  (per-position
  head-Gram matrices), softmax over s (the sequence!), and the output einsum factorizes:
     out[n,qh,s,d] = A[n,qh,s] * Vs[n,s,d]
  with A = sum_kh softmax_s(energy/8) and Vs = sum_vh v.  The (n,qh,s,d) tensor is then
  C-order reshaped to (n, s', e') with s' = qh*128 + s//16, e' = (s%16)*64 + (d) and fed
  through Wo + residual + LN1 + FFN + LN2.

Sharding: 8 cores, core c handles batch n=c//2 and head-half hs=c%2 (qh in [8hs, 8hs+8)),
which makes output rows t' in [1024hs, 1024hs+1024) — fully local, zero collectives.
"""

import sys

sys.path.insert(0, "/opt/trn_rl_repo")

from contextlib import ExitStack

import numpy as np
import ml_dtypes

import concourse.bass as bass
from concourse import bacc
import concourse.mybir as mybir
import concourse.tile as tile
from concourse.bass import ts
from concourse.bass_utils import run_bass_kernel_spmd

F32 = mybir.dt.float32
F32R = mybir.dt.float32r
BF16 = mybir.dt.bfloat16
AF = mybir.ActivationFunctionType
ALU = mybir.AluOpType

E = 1024
H = 16
D = 64
F = 4096
NB = 4
S = 2048
TLOC = 1024  # output rows per core
EPS = 1e-5
NCORES = 8

_BF = ml_dtypes.bfloat16

_nc_cache = {}


# ---------------------------------------------------------------------------
# device program
# ---------------------------------------------------------------------------

def _declare(nc):
    dp = nc.declare_dram_parameter
    t = {}
    t["xth"] = dp("xth", [128, 8, S], BF16, isOutput=False)        # x[n].T chunked, bf16
    t["xtr"] = dp("xtr", [128, 8, TLOC], F32, isOutput=False)      # residual slice of x
    t["wqr"] = dp("wqr", [8, 128, 4, 256], BF16, isOutput=False)  # [qh][p, e2, k*128+i]
    t["wk"] = dp("wk", [128, 8, E], BF16, isOutput=False)
    t["wvs"] = dp("wvs", [128, 8, 256], BF16, isOutput=False)      # [p, ki, l|r]
    t["wo"] = dp("wo", [128, 8, E], BF16, isOutput=False)
    t["w1"] = dp("w1", [8, 128, 8, 512], BF16, isOutput=False)     # [c][p, ki, j*128+m]
    t["w2"] = dp("w2", [8, 128, 16, 256], BF16, isOutput=False)    # [mc][p, i, k*128+m]
    t["maske"] = dp("maske", [128, 8, 32], BF16, isOutput=False)
    t["maskq"] = dp("maskq", [128, 2, 8], F32, isOutput=False)
    t["bc2"] = dp("bc2", [2, 128], BF16, isOutput=False)
    t["onesk"] = dp("onesk", [128, 1], F32R, isOutput=False)
    t["onesm"] = dp("onesm", [1, 128], F32R, isOutput=False)
    t["bo"] = dp("bo", [128, 8], F32, isOutput=False)
    t["bf1"] = dp("bf1", [128, 32], F32, isOutput=False)
    t["bf2"] = dp("bf2", [128, 8], F32, isOutput=False)
    t["ln1w"] = dp("ln1w", [128, 8], F32, isOutput=False)
    t["ln1b"] = dp("ln1b", [128, 8], F32, isOutput=False)
    t["ln2w"] = dp("ln2w", [128, 8], F32, isOutput=False)
    t["ln2b"] = dp("ln2b", [128, 8], F32, isOutput=False)
    t["out"] = dp("out", [E, TLOC], F32, isOutput=True)            # y2 transposed [e, t']
    return t


class _LnState:
    """Interleaved layernorm: stats matmuls are emitted chunk-by-chunk from the
    producer loop; finalize computes mean/rstd, broadcasts, and normalizes."""

    def __init__(self, nc, pps, spool, onesk, tag):
        self.nc = nc
        self.pps = pps
        self.spool = spool
        self.onesk = onesk
        self.tag = tag
        self.s_ps = [pps.tile([1, 512], F32, tag=f"{tag}_s", name=f"{tag}_s{w}") for w in range(2)]
        self.q_ps = [pps.tile([1, 512], F32, tag=f"{tag}_q", name=f"{tag}_q{w}") for w in range(2)]

    def stats_chunk(self, srcs, mc):
        """srcs: list of 8 SBUF [128, 1024] F32R chunk tiles; emit stat MMs for chunk mc."""
        nc = self.nc
        for w2 in range(2):
            sl = srcs[mc][:, ts(w2, 512)]
            nc.tensor.matmul(self.s_ps[w2][:], self.onesk[:], sl, start=(mc == 0), stop=(mc == 7))
            sq = self.spool.tile([128, 512], F32R, tag=f"{self.tag}_sq")
            nc.scalar.activation(out=sq[:], in_=sl.bitcast(F32), func=AF.Square)
            nc.tensor.matmul(self.q_ps[w2][:], self.onesk[:], sq[:], start=(mc == 0), stop=(mc == 7))


def _ln_finalize(nc, st, pools, srcs, dst_write, wcol, bcol, eps_sb, tag, chunk_done=None):
    """Normalize using stats accumulated in st.  dst_write(mc, w2) -> AP [128, 512].

    mc-outer so consumers that read chunk mc across both halves unblock early.
    The mean/rstd broadcast tiles for the two halves steal distinct drained
    PSUM rings (ln_bcast_pool{,2}) so the halves don't serialize on WAR deps.
    """
    spool = pools["ln_sbuf"]
    rpool = pools["ln_row"]
    bcast = []
    for w2 in range(2):
        br_pool = pools["ln_bcast_pool"] if w2 == 0 else pools["ln_bcast_pool2"]
        br_tag = pools["ln_bcast_tag"] if w2 == 0 else pools["ln_bcast_tag2"]
        mean = rpool.tile([1, 512], F32R, tag=f"{tag}_mean", name=f"{tag}_mean{w2}")
        msq = rpool.tile([1, 512], F32, tag=f"{tag}_msq", name=f"{tag}_msq{w2}")
        nc.scalar.activation(out=mean[:], in_=st.s_ps[w2][:], func=AF.Copy, scale=1.0 / E)
        nc.scalar.activation(out=msq[:], in_=st.q_ps[w2][:], func=AF.Copy, scale=1.0 / E)
        m2 = rpool.tile([1, 512], F32, tag=f"{tag}_m2", name=f"{tag}_m2{w2}")
        nc.vector.tensor_mul(m2[:], mean[:].bitcast(F32), mean[:].bitcast(F32))
        var = rpool.tile([1, 512], F32, tag=f"{tag}_var", name=f"{tag}_var{w2}")
        nc.vector.tensor_tensor(var[:], msq[:], m2[:], ALU.subtract)
        # sd/rs reuse the msq/m2 rings (dead after the var computation)
        sd = rpool.tile([1, 512], F32, tag=f"{tag}_msq", name=f"{tag}_sd{w2}")
        nc.scalar.activation(out=sd[:], in_=var[:], func=AF.Sqrt, bias=eps_sb[:])
        rs = rpool.tile([1, 512], F32R, tag=f"{tag}_m2", name=f"{tag}_rs{w2}")
        with nc.allow_low_precision(reason="f32r is ample for the LN rstd row"):
            nc.vector.reciprocal(rs[:], sd[:])
        mb_ps = br_pool.tile([128, 512], F32, tag=br_tag, name=f"{tag}_mb{w2}")
        rb_ps = br_pool.tile([128, 512], F32, tag=br_tag, name=f"{tag}_rb{w2}")
        nc.tensor.matmul(mb_ps[:], pools["onesm"][:], mean[:], start=True, stop=True)
        nc.tensor.matmul(rb_ps[:], pools["onesm"][:], rs[:], start=True, stop=True)
        # drain to SBUF: GpSimd cannot read PSUM, and SBUF is faster for DVE too
        mb_sb = spool.tile([128, 512], F32, tag=f"{tag}_mbs", name=f"{tag}_mbs{w2}")
        rb_sb = spool.tile([128, 512], F32, tag=f"{tag}_rbs", name=f"{tag}_rbs{w2}")
        nc.scalar.copy(mb_sb[:], mb_ps[:])
        nc.scalar.copy(rb_sb[:], rb_ps[:])
        bcast.append((mb_sb, rb_sb))
    for mc in range(8):
        for w2 in range(2):
            mb_sb, rb_sb = bcast[w2]
            # spread the normalize chains over DVE and the otherwise-idle
            # GpSimd engine (~1.7:1 throughput ratio -> 6-of-16 chunks on Pool);
            # the affine (w,b) step runs on the Activation engine
            on_pool = (2 * mc + w2) % 8 in (2, 5, 7)
            eng = nc.gpsimd if on_pool else nc.vector
            sub = "p" if on_pool else "v"  # per-engine rings so the pipelines decouple
            t1 = spool.tile([128, 512], F32, tag=f"{tag}_t1{sub}")
            eng.tensor_tensor(t1[:], srcs[mc][:, ts(w2, 512)].bitcast(F32), mb_sb[:], ALU.subtract)
            t2 = spool.tile([128, 512], F32, tag=f"{tag}_t2{sub}")
            eng.tensor_mul(t2[:], t1[:], rb_sb[:])
            nc.scalar.activation(
                out=dst_write(mc, w2),
                in_=t2[:],
                func=AF.Identity,
                scale=wcol[:, mc : mc + 1],
                bias=bcol[:, mc : mc + 1],
            )
        if chunk_done is not None:
            chunk_done(mc)


def _emit(nc, t, reps=1):
    with tile.TileContext(nc) as tc:
      for _rep in range(reps):
        with ExitStack() as octx:
          pool_const = octx.enter_context(tc.tile_pool(name="const", bufs=1))
          pool_y1 = octx.enter_context(tc.tile_pool(name="y1p", bufs=1))
          # opened at top level so its SBUF is disjoint from the attention
          # phase pools: the first w1 prefetch DMA must not WAR-wait on them
          pool_w1 = octx.enter_context(tc.tile_pool(name="pw1", bufs=2))
          y1c = [pool_y1.tile([128, TLOC], BF16, tag=f"y1_{mc}", name=f"y1_{mc}") for mc in range(8)]

          with ExitStack() as actx:
              pool_att = actx.enter_context(tc.tile_pool(name="att", bufs=1))
              u_sb = pool_att.tile([128, 2, S], F32R, tag="u")
              vsh_sb = pool_att.tile([128, S], F32, tag="vsh")
              dsum_sb = pool_att.tile([128, 8], F32, tag="dsum")
              dden_sb = pool_att.tile([128, 2], F32, tag="dden")
              rec_sb = pool_att.tile([128, 2], F32, tag="rec")
              v_sb = pool_att.tile([128, 2, 8], F32R, tag="vv")
              a_sb = pool_att.tile([8, 16, 128], BF16, tag="aa")  # [qh, r, sb]
              arow_sb = pool_att.tile([2, 8, 8, 128], BF16, tag="arow")  # [j, qh, pc, sb]

              # ================= phases B/C/D: projections + energy =================
              with ExitStack() as pctx:
                  pool_x = pctx.enter_context(tc.tile_pool(name="px", bufs=1))
                  pool_wq = pctx.enter_context(tc.tile_pool(name="pwq", bufs=2))
                  pool_kt = pctx.enter_context(tc.tile_pool(name="pkt", bufs=1))
                  pool_str = pctx.enter_context(tc.tile_pool(name="pstr", bufs=3))
                  ps_pp = pctx.enter_context(tc.tile_pool(name="pspp", bufs=4, space="PSUM"))
                  ps_e = pctx.enter_context(tc.tile_pool(name="pse", bufs=4, space="PSUM"))

                  # critical-path DMAs first: wk half 0, then x chunks, then the rest
                  wk_sb = pool_x.tile([128, 8, E], BF16, tag="wk")
                  xth_sb = pool_x.tile([128, 8, S + 1], BF16, tag="xth")
                  nc.sync.dma_start(wk_sb[:, :, 0:512], t["wk"].ap()[:, :, 0:512])
                  for w in range(4):
                      nc.sync.dma_start(
                          xth_sb[:, :, ts(w, 512)], t["xth"].ap()[:, :, ts(w, 512)]
                      )
                  nc.sync.dma_start(wk_sb[:, :, 512:1024], t["wk"].ap()[:, :, 512:1024])
                  wvs_sb = pool_x.tile([128, 8, 256], BF16, tag="wvs")
                  nc.sync.dma_start(wvs_sb[:], t["wvs"].ap()[:, :, :])
                  nc.vector.memset(xth_sb[:, :, S : S + 1], 0.0)

                  # ---- constants (after the critical path) ----
                  maske_sb = pool_const.tile([128, 8, 32], BF16, tag="maske")
                  nc.sync.dma_start(maske_sb[:], t["maske"].ap()[:, :, :])
                  maskq_sb = pool_const.tile([128, 2, 8], F32, tag="maskq")
                  nc.sync.dma_start(maskq_sb[:], t["maskq"].ap()[:, :, :])
                  bc2_sb = pool_const.tile([2, 128], BF16, tag="bc2")
                  nc.sync.dma_start(bc2_sb[:], t["bc2"].ap()[:, :])
                  onesk_sb = pool_const.tile([128, 1], F32R, tag="onesk")
                  nc.sync.dma_start(onesk_sb[:], t["onesk"].ap()[:, :])
                  onesm_sb = pool_const.tile([1, 128], F32R, tag="onesm")
                  nc.sync.dma_start(onesm_sb[:], t["onesm"].ap()[:, :])
                  cols = {}
                  for nm in ("bo", "bf1", "bf2", "ln1w", "ln1b", "ln2w", "ln2b"):
                      sh = [128, 32] if nm == "bf1" else [128, 8]
                      cols[nm] = pool_const.tile(sh, F32, tag=nm, name=nm)
                      nc.sync.dma_start(cols[nm][:], t[nm].ap()[:, :])
                  eps_sb = pool_const.tile([1, 1], F32, tag="eps")
                  nc.vector.memset(eps_sb[:], EPS)

                  kt_sb = pool_kt.tile([128, 8, S], BF16, tag="kt")

                  # ---- B: k projection (all 16 heads); ki outer so lhsT is reused ----
                  for ek in range(8):
                      kps = [ps_pp.tile([128, 512], F32, tag="pp", name=f"kp{ek}_{i}") for i in range(4)]
                      for ki in range(8):
                          for w in range(4):
                              nc.tensor.matmul(
                                  kps[w][:],
                                  wk_sb[:, ki, ts(ek, 128)],
                                  xth_sb[:, ki, ts(w, 512)],
                                  start=(ki == 0),
                                  stop=(ki == 7),
                              )
                      for w in range(4):
                          nc.scalar.copy(kt_sb[:, ek, ts(w, 512)], kps[w][:])

                  # ---- D: shifted Vs projection (vsh[i, s] = Vs[s + i//64, i%64]) ----
                  vps = [ps_pp.tile([128, 512], F32, tag="pp", name=f"vp{i}") for i in range(4)]
                  for ki in range(8):
                      for w in range(4):
                          nc.tensor.matmul(
                              vps[w][:],
                              wvs_sb[:, ki, 0:128],
                              xth_sb[:, ki, ts(w, 512)],
                              start=(ki == 0),
                              stop=False,
                          )
                  for ki in range(8):
                      for w in range(4):
                          nc.tensor.matmul(
                              vps[w][:],
                              wvs_sb[:, ki, 128:256],
                              xth_sb[:, ki, w * 512 + 1 : w * 512 + 513],
                              start=False,
                              stop=(ki == 7),
                          )
                  for w in range(4):
                      nc.scalar.copy(vsh_sb[:, ts(w, 512)], vps[w][:])

                  # ---- C: qrep2 + products + energy block-reduce + exp ----
                  for h in range(2):
                      e_tiles = [ps_e.tile([128, 512], F32, tag="e", name=f"e{h}_{i}") for i in range(4)]
                      for jq in range(4):
                          qh = 4 * h + jq
                          wqr_t = pool_wq.tile([128, 4, 256], BF16, tag="wqrt")
                          nc.sync.dma_start(wqr_t[:], t["wqr"].ap()[qh, :, :, :])
                          qr_sb = pool_str.tile([128, S], BF16, tag="qr")
                          qps = [ps_pp.tile([128, 512], F32, tag="pp", name=f"qp{qh}_{i}") for i in range(4)]
                          for ki in range(8):
                              for w in range(4):
                                  nc.tensor.matmul(
                                      qps[w][:],
                                      wqr_t[:, ki // 2, ts(ki % 2, 128)],
                                      xth_sb[:, ki, ts(w, 512)],
                                      start=(ki == 0),
                                      stop=(ki == 7),
                                  )
                          for w in range(4):
                              nc.scalar.copy(qr_sb[:, ts(w, 512)], qps[w][:])
                          for ek in range(8):
                              z = pool_str.tile([128, S], BF16, tag="z")
                              nc.vector.tensor_mul(z[:], kt_sb[:, ek, :], qr_sb[:])
                              for w in range(4):
                                  nc.tensor.matmul(
                                      e_tiles[w][32 * jq : 32 * jq + 32, :],
                                      maske_sb[:, ek, :],
                                      z[:, ts(w, 512)],
                                      start=(ek == 0),
                                      stop=(ek == 7),
                                      tile_position=(0, 32 * jq),
                                      skip_group_check=True,
                                  )
                      for w in range(4):
                          nc.scalar.activation(
                              out=u_sb[:, h, ts(w, 512)],
                              in_=e_tiles[w][:],
                              func=AF.Exp,
                              scale=0.125,
                              accum_out=dsum_sb[:, 4 * h + w : 4 * h + w + 1],
                          )

              # ================= phase E: denominators + A =================
              with ExitStack() as ectx:
                  ps_a = ectx.enter_context(tc.tile_pool(name="psa", bufs=2, space="PSUM"))
                  for h in range(2):
                      nc.vector.reduce_sum(
                          out=dden_sb[:, h : h + 1],
                          in_=dsum_sb[:, ts(h, 4)],
                          axis=mybir.AxisListType.X,
                      )
                      nc.vector.reciprocal(rec_sb[:, h : h + 1], dden_sb[:, h : h + 1])
                      nc.vector.tensor_scalar_mul(
                          out=v_sb[:, h, :], in0=maskq_sb[:, h, :], scalar1=rec_sb[:, h : h + 1]
                      )
                  for w in range(4):
                      a_ps = ps_a.tile([8, 512], F32, tag="a")
                      nc.tensor.matmul(a_ps[:], v_sb[:, 0, :], u_sb[:, 0, ts(w, 512)], start=True, stop=False)
                      nc.tensor.matmul(a_ps[:], v_sb[:, 1, :], u_sb[:, 1, ts(w, 512)], start=False, stop=True)
                      # scatter-write A window into [qh, r, sb] layout:
                      # in t-order (sb outer, r inner) -> out dims (sb, r)
                      nc.scalar.copy(
                          a_sb[:, :, 32 * w : 32 * w + 32].transpose([0, 2, 1]), a_ps[:]
                      )

                  # A gather: arow[j, qh, pc, sb] = a_sb[qh, 2*pc+j, sb]
                  with nc.allow_non_contiguous_dma(reason="tiny A gather"):
                      for j in range(2):
                          nc.sync.dma_start(
                              arow_sb[j : j + 1, :, :, :],
                              a_sb[:, j::2, :],
                          )

              # ================= phase F: OFT build =================
              with ExitStack() as fctx:
                  pool_oft = fctx.enter_context(tc.tile_pool(name="poft", bufs=1))
                  oftc = [pool_oft.tile([128, TLOC], BF16, tag=f"oft{pc}", name=f"oft{pc}") for pc in range(8)]
                  with ExitStack() as ffctx:
                      ps_fa = ffctx.enter_context(tc.tile_pool(name="psfa", bufs=2, space="PSUM"))
                      for pc in range(8):
                          for w2 in range(2):
                              fa_ps = ps_fa.tile([128, 512], F32, tag="fa")
                              nc.tensor.matmul(
                                  fa_ps[:],
                                  bc2_sb[:],
                                  arow_sb[:, ts(w2, 4), pc, :],
                                  start=True,
                                  stop=True,
                              )
                              nc.vector.tensor_mul(
                                  oftc[pc][:, ts(w2, 512)].rearrange("p (a b) -> p a b", a=4),
                                  fa_ps[:].rearrange("p (a b) -> p a b", a=4),
                                  vsh_sb[:, 2 * pc :: 16].unsqueeze(1).broadcast_to([128, 4, 128]),
                              )

                  # ================= phase G: Wo + residual + LN1 =================
                  with ExitStack() as gctx:
                          pool_g = gctx.enter_context(tc.tile_pool(name="pg", bufs=1))
                          pool_gx = gctx.enter_context(tc.tile_pool(name="pgx", bufs=2))
                          pool_gt = gctx.enter_context(tc.tile_pool(name="pgt", bufs=2))
                          pool_grow = gctx.enter_context(tc.tile_pool(name="pgrow", bufs=1))
                          ps_g = gctx.enter_context(tc.tile_pool(name="psg", bufs=4, space="PSUM"))
                          ps_ln1 = gctx.enter_context(tc.tile_pool(name="psln1", bufs=2, space="PSUM"))

                          wo_sb = pool_g.tile([128, 8, E], BF16, tag="wo")
                          nc.sync.dma_start(wo_sb[:], t["wo"].ap()[:, :, :])
                          y1pc = [pool_g.tile([128, TLOC], F32R, tag=f"y1pre{mc}", name=f"y1pre{mc}") for mc in range(8)]

                          ln1 = _LnState(nc, ps_ln1, pool_gt, onesk_sb, "ln1")

                          for mc in range(8):
                              xtr_t = pool_gx.tile([128, TLOC], F32, tag="xtrt", name=f"xtr{mc}")
                              nc.sync.dma_start(xtr_t[:], t["xtr"].ap()[:, mc, :])
                              at_tiles = [
                                  ps_g.tile([128, 512], F32, tag="at", name=f"at{mc}_{i}")
                                  for i in range(2)
                              ]
                              for pc in range(8):
                                  for w2 in range(2):
                                      nc.tensor.matmul(
                                          at_tiles[w2][:],
                                          wo_sb[:, pc, ts(mc, 128)],
                                          oftc[pc][:, ts(w2, 512)],
                                          start=(pc == 0),
                                          stop=(pc == 7),
                                      )
                              for w2 in range(2):
                                  attn = pool_gt.tile([128, 512], F32, tag="attn", name=f"attn{mc}_{w2}")
                                  nc.scalar.activation(
                                      out=attn[:], in_=at_tiles[w2][:], func=AF.Identity, bias=cols["bo"][:, mc : mc + 1]
                                  )
                                  nc.vector.tensor_add(
                                      y1pc[mc][:, ts(w2, 512)], attn[:], xtr_t[:, ts(w2, 512)]
                                  )
                              if mc >= 1:
                                  ln1.stats_chunk(y1pc, mc - 1)
                          ln1.stats_chunk(y1pc, 7)

                          def dst1(mc, w2):
                              return y1c[mc][:, ts(w2, 512)]

                          ln_pools = {
                              "ln_sbuf": pool_gt,
                              "ln_row": pool_grow,
                              "ln_bcast_pool": ps_g,
                              "ln_bcast_tag": "at",
                              "ln_bcast_pool2": ps_g,
                              "ln_bcast_tag2": "at",
                              "onesm": onesm_sb,
                          }
                          _ln_finalize(
                              nc, ln1, ln_pools,
                              srcs=y1pc,
                              dst_write=dst1,
                              wcol=cols["ln1w"], bcol=cols["ln1b"], eps_sb=eps_sb, tag="ln1",
                          )

          # ================= phase H: FFN + LN2 + out =================
          with ExitStack() as hctx:
              pool_h = hctx.enter_context(tc.tile_pool(name="ph", bufs=1))
              pool_w2 = hctx.enter_context(tc.tile_pool(name="pw2", bufs=2))
              pool_ht = hctx.enter_context(tc.tile_pool(name="pht", bufs=2))
              pool_hrow = hctx.enter_context(tc.tile_pool(name="phrow", bufs=1))
              pool_y2 = hctx.enter_context(tc.tile_pool(name="py2", bufs=2))
              ps_h1 = hctx.enter_context(tc.tile_pool(name="psh1", bufs=2, space="PSUM"))
              ps_h2 = hctx.enter_context(tc.tile_pool(name="psh2", bufs=2, space="PSUM"))
              ps_ln2 = hctx.enter_context(tc.tile_pool(name="psln2", bufs=2, space="PSUM"))

              hc = [pool_h.tile([128, TLOC], BF16, tag=f"hh{fc}", name=f"hh{fc}") for fc in range(32)]
              ffc = [pool_h.tile([128, TLOC], F32R, tag=f"ffpre{mc}", name=f"ffpre{mc}") for mc in range(8)]

              # prefetch first w2 chunk before FFN1 so FFN2 starts without a stall
              w2t_first = pool_w2.tile([128, 16, 256], BF16, tag="w2t", name="w2t0")
              nc.sync.dma_start(w2t_first[:], t["w2"].ap()[0, :, :, :])

              # ---- FFN1: single pass over both token halves ----
              for c in range(8):
                  w1t = pool_w1.tile([128, 8, 512], BF16, tag="w1t", name=f"w1t{c}")
                  nc.sync.dma_start(w1t[:], t["w1"].ap()[c, :, :, :])
                  for j in range(4):
                      fc = 4 * c + j
                      h_ps = [
                          ps_h1.tile([128, 512], F32, tag="h", name=f"h{fc}_{w2}")
                          for w2 in range(2)
                      ]
                      for ki in range(8):
                          for w2 in range(2):
                              nc.tensor.matmul(
                                  h_ps[w2][:],
                                  w1t[:, ki, ts(j, 128)],
                                  y1c[ki][:, ts(w2, 512)],
                                  start=(ki == 0),
                                  stop=(ki == 7),
                              )
                      for w2 in range(2):
                          nc.scalar.activation(
                              out=hc[fc][:, ts(w2, 512)],
                              in_=h_ps[w2][:],
                              func=AF.Relu,
                              bias=cols["bf1"][:, fc : fc + 1],
                          )

              # ---- FFN2 + residual + interleaved LN2 stats ----
              ln2 = _LnState(nc, ps_ln2, pool_ht, onesk_sb, "ln2")
              for mc in range(8):
                  if mc == 0:
                      w2t = w2t_first
                  else:
                      w2t = pool_w2.tile([128, 16, 256], BF16, tag="w2t", name=f"w2t{mc}")
                      nc.sync.dma_start(w2t[:], t["w2"].ap()[mc, :, :, :])
                  ff_ps = [
                      ps_h2.tile([128, 512], F32, tag="ff", name=f"ff{mc}_{w2}")
                      for w2 in range(2)
                  ]
                  for fc in range(32):
                      for w2 in range(2):
                          nc.tensor.matmul(
                              ff_ps[w2][:],
                              w2t[:, fc // 2, ts(fc % 2, 128)],
                              hc[fc][:, ts(w2, 512)],
                              start=(fc == 0),
                              stop=(fc == 31),
                          )
                  for w2 in range(2):
                      fft = pool_ht.tile([128, 512], F32, tag="fft", name=f"fft{mc}_{w2}")
                      nc.scalar.activation(
                          out=fft[:], in_=ff_ps[w2][:], func=AF.Identity, bias=cols["bf2"][:, mc : mc + 1]
                      )
                      nc.vector.tensor_add(
                          ffc[mc][:, ts(w2, 512)], fft[:], y1c[mc][:, ts(w2, 512)]
                      )
                  if mc >= 1:
                      ln2.stats_chunk(ffc, mc - 1)
              ln2.stats_chunk(ffc, 7)

              out_ap = t["out"].ap()
              y2_tiles = {}

              def dst2(mc, w2):
                  wt = pool_y2.tile([128, 512], F32, tag="y2out", name=f"y2_{mc}_{w2}")
                  y2_tiles[(mc, w2)] = wt
                  return wt[:]

              ln_pools = {
                  "ln_sbuf": pool_ht,
                  "ln_row": pool_hrow,
                  "ln_bcast_pool": ps_h1,
                  "ln_bcast_tag": "h",
                  "ln_bcast_pool2": ps_h2,
                  "ln_bcast_tag2": "ff",
                  "onesm": onesm_sb,
              }
              _ln_finalize(
                  nc, ln2, ln_pools,
                  srcs=ffc,
                  dst_write=dst2,
                  wcol=cols["ln2w"], bcol=cols["ln2b"], eps_sb=eps_sb, tag="ln2",
                  chunk_done=lambda mc: [
                      nc.sync.dma_start(
                          out_ap[ts(mc, 128), ts(w2, 512)], y2_tiles[(mc, w2)][:]
                      )
                      for w2 in range(2)
                  ],
              )


def build_nc(reps=1):
    key = f"nc{reps}"
    if key in _nc_cache:
        return _nc_cache[key]
    nc = bacc.Bacc("TRN2", target_bir_lowering=False, debug=False, num_devices=NCORES)
    t = _declare(nc)
    _emit(nc, t, reps=reps)
    nc.compile()
    _nc_cache[key] = nc
    return nc


# ---------------------------------------------------------------------------
# host side
# ---------------------------------------------------------------------------

def _chunk_pT(a):
    """[R, E] -> [128, E//128, R]: out[p, ek, r] = a[r, ek*128+p]."""
    r, e = a.shape
    return np.ascontiguousarray(a.reshape(r, e // 128, 128).transpose(2, 1, 0))


def _cols(b, nchunk):
    """[C] -> [128, nchunk]: out[p, c] = b[c*128+p]."""
    return np.ascontiguousarray(b.reshape(nchunk, 128).T)


def prepare_inputs(x, Wq, Wk, Wv, Wo, bo, ln1_w, ln1_b, ln2_w, ln2_b, W1, bf1, W2, bf2):
    """Build the 8 per-core input maps."""
    x = np.asarray(x, dtype=np.float32)
    Wq = np.asarray(Wq, dtype=np.float32)
    Wk = np.asarray(Wk, dtype=np.float32)
    Wv = np.asarray(Wv, dtype=np.float32)
    Wo = np.asarray(Wo, dtype=np.float32)
    W1 = np.asarray(W1, dtype=np.float32)
    W2 = np.asarray(W2, dtype=np.float32)
    bo = np.asarray(bo, dtype=np.float32)
    bf1 = np.asarray(bf1, dtype=np.float32)
    bf2 = np.asarray(bf2, dtype=np.float32)
    ln1_w = np.asarray(ln1_w, dtype=np.float32)
    ln1_b = np.asarray(ln1_b, dtype=np.float32)
    ln2_w = np.asarray(ln2_w, dtype=np.float32)
    ln2_b = np.asarray(ln2_b, dtype=np.float32)

    # shared (core-independent) tensors
    wk_h = _chunk_pT(Wk).astype(_BF)                      # [128, 8, 1024]
    Wvs = Wv.reshape(H, D, E).sum(axis=0)                 # [64, 1024]
    blk = _chunk_pT(Wvs)                                  # [128, 8, 64]
    z64 = np.zeros_like(blk)
    wvsl_h = np.concatenate([blk, z64], axis=2)           # [128, 8, 128]
    wvsr_h = np.concatenate([z64, blk], axis=2)
    wvs_h = np.concatenate([wvsl_h, wvsr_h], axis=2).astype(_BF)  # [128, 8, 256]
    wo_h = _chunk_pT(Wo).astype(_BF)
    # w1: [c][p, ki, j*128+m] = W1[(4c+j)*128+m, ki*128+p]
    w1_h = np.ascontiguousarray(
        W1.reshape(8, 4, 128, 8, 128).transpose(0, 4, 3, 1, 2).reshape(8, 128, 8, 512)
    ).astype(_BF)
    # w2: [mc][p, i, k*128+m] = W2[mc*128+m, (2i+k)*128+p]
    w2_h = np.ascontiguousarray(
        W2.reshape(8, 128, 16, 2, 128).transpose(0, 4, 2, 3, 1).reshape(8, 128, 16, 256)
    ).astype(_BF)

    i128 = np.arange(128)
    maske = np.zeros((128, 8, 32), dtype=np.float32)
    for c in range(8):
        maske[i128, c, (2 * c + i128 // 64)] = 1.0
    maske = maske.astype(_BF)
    # half hh covers local qh' in [4*hh, 4*hh+4); pair (qh', kh) sits at
    # partition p = 32*(qh' % 4) + kh (kh < 16).  A-matmul column = qh'.
    maskq = np.zeros((128, 2, 8), dtype=np.float32)
    for hh in range(2):
        for p in range(128):
            if p % 32 < 16:
                maskq[p, hh, 4 * hh + p // 32] = 1.0
    bc2 = np.zeros((2, 128), dtype=np.float32)
    bc2[0, :64] = 1.0
    bc2[1, 64:] = 1.0
    bc2 = bc2.astype(_BF)
    onesk = np.ones((128, 1), dtype=np.float32)
    onesm = np.ones((1, 128), dtype=np.float32)

    shared = {
        "wk": wk_h, "wvs": wvs_h, "wo": wo_h,
        "w1": w1_h, "w2": w2_h,
        "maske": maske, "maskq": maskq, "bc2": bc2,
        "onesk": onesk, "onesm": onesm,
        "bo": _cols(bo, 8), "bf1": _cols(bf1, 32), "bf2": _cols(bf2, 8),
        "ln1w": _cols(ln1_w, 8), "ln1b": _cols(ln1_b, 8),
        "ln2w": _cols(ln2_w, 8), "ln2b": _cols(ln2_b, 8),
    }

    in_maps = []
    for c in range(NCORES):
        n, hs = c // 2, c % 2
        xn = x[n]                                          # [2048, 1024]
        xth = _chunk_pT(xn).astype(_BF)                    # [128, 8, 2048]
        xtr = _chunk_pT(xn[hs * TLOC : (hs + 1) * TLOC])   # [128, 8, 1024] f32
        Wqh = Wq[hs * 512 : (hs + 1) * 512]                # [512, 1024]
        tmp = Wqh.reshape(8, D, 8, 128)                    # [qh, d, ek, p]
        dup = np.concatenate([tmp, tmp], axis=1)           # [qh, 128, ek, p]
        wqr = np.ascontiguousarray(dup.transpose(0, 3, 2, 1)).astype(_BF)  # [qh, 128, ek, 128]
        wqr = wqr.reshape(8, 128, 4, 256)                  # pack ek pairs: 512B runs
        m = dict(shared)
        m["xth"] = xth
        m["xtr"] = xtr
        m["wqr"] = wqr
        in_maps.append(m)
    return in_maps


def assemble_output(results):
    out = np.empty((NB, S, E), dtype=np.float32)
    for c in range(NCORES):
        n, hs = c // 2, c % 2
        out[n, hs * TLOC : (hs + 1) * TLOC, :] = results[c]["out"].T
    return out


# ---------------------------------------------------------------------------
# cached PJRT runner (jit built once; shared weights replicated, not concatenated)
# ---------------------------------------------------------------------------

_PER_CORE = ("xth", "xtr", "wqr")  # inputs that differ per core


def _get_runner():
    if "runner" in _nc_cache:
        return _nc_cache["runner"]
    import jax
    from jax.sharding import Mesh, PartitionSpec
    from jax.experimental.shard_map import shard_map
    from concourse import bass2jax, mybir as _mb

    nc = build_nc()
    bass2jax.install_neuronx_cc_hook()

    partition_name = nc.partition_id_tensor.name if nc.partition_id_tensor else None
    in_names, out_names, out_avals, zero_shapes = [], [], [], []
    for alloc in nc.m.functions[0].allocations:
        if not isinstance(alloc, _mb.MemoryLocationSet):
            continue
        name = alloc.memorylocations[0].name
        if alloc.kind == "ExternalInput":
            if name != partition_name:
                in_names.append(name)
        elif alloc.kind == "ExternalOutput":
            out_names.append(name)
            shape = tuple(alloc.tensor_shape)
            dtype = _mb.dt.np(alloc.dtype)
            out_avals.append(jax.core.ShapedArray(shape, dtype))
            zero_shapes.append((shape, dtype))
    n_params = len(in_names)
    all_names = list(in_names) + out_names
    if partition_name is not None:
        all_names.append(partition_name)

    def _body(*args):
        operands = list(args)
        if partition_name is not None:
            operands.append(bass2jax.partition_id_tensor())
        outs = bass2jax._bass_exec_p.bind(
            *operands,
            out_avals=tuple(out_avals),
            in_names=tuple(all_names),
            out_names=tuple(out_names),
            lowering_input_output_aliases=(),
            sim_require_finite=True,
            sim_require_nnan=True,
            nc=nc,
        )
        return tuple(outs)

    devices = jax.devices()[:NCORES]
    mesh = Mesh(np.asarray(devices), ("core",))
    in_specs = tuple(
        PartitionSpec("core") if nm in _PER_CORE else PartitionSpec()
        for nm in in_names
    ) + (PartitionSpec("core"),) * len(out_names)
    out_specs = (PartitionSpec("core"),) * len(out_names)
    donate = tuple(range(n_params, n_params + len(out_names)))
    sharded = jax.jit(
        shard_map(_body, mesh=mesh, in_specs=in_specs, out_specs=out_specs, check_rep=False),
        donate_argnums=donate,
        keep_unused=True,
    )
    runner = {
        "jit": sharded,
        "in_names": in_names,
        "out_names": out_names,
        "zero_shapes": zero_shapes,
        "out_avals": out_avals,
        "mesh": mesh,
        "in_specs": in_specs,
    }
    _nc_cache["runner"] = runner
    return runner


def _pack_args(runner, in_maps):
    """Build the positional arg list for the jitted runner from per-core maps."""
    args = []
    for nm in runner["in_names"]:
        if nm in _PER_CORE:
            args.append(np.concatenate([np.asarray(m[nm]) for m in in_maps], axis=0))
        else:
            args.append(np.asarray(in_maps[0][nm]))
    return args


def _zero_outs(runner):
    return [
        np.zeros((NCORES * sh[0], *sh[1:]), dt) for sh, dt in runner["zero_shapes"]
    ]


def run_fast(in_maps):
    runner = _get_runner()
    args = _pack_args(runner, in_maps)
    outs = runner["jit"](*args, *_zero_outs(runner))
    res = []
    for c in range(NCORES):
        m = {}
        for i, nm in enumerate(runner["out_names"]):
            sh = runner["out_avals"][i].shape
            m[nm] = np.asarray(outs[i]).reshape(NCORES, *sh)[c]
        res.append(m)
    return res


def kernel(**inputs):
    in_maps = prepare_inputs(**inputs)
    return assemble_output(run_fast(in_maps))


if __name__ == "__main__":
    # quick self-build check
    nc = build_nc()
    print("built ok")
